# revision 32
# baseline (speedup 1.0000x reference)
"""GGNN (gated graph NN) message-passing kernel for Trainium2, 8 NeuronCores.

Model (per reference):
  5 steps of: s_in = state @ in_W.T + in_b ; s_out = state @ out_W.T + out_b
              a_in = A @ s_in ; a_out = A.T @ s_out
              r = sigmoid([a_in, a_out, state] @ r_W.T + r_b)
              z = sigmoid([a_in, a_out, state] @ z_W.T + z_b)
              h = tanh([a_in, a_out, r*state] @ h_W.T + h_b)
              state = (1-z)*state + z*h
  out = tanh(state @ o1_W.T + o1_b) @ o2_W.T + o2_b

Sharding: 1D node parallelism over 8 cores (512 nodes each). Weights
replicated. Per step, each core computes its s_in/s_out shard, all-gathers
them, then computes its row-shard of the aggregations and gates locally.

Precision: the recurrence amplifies operand rounding ~2500x, so bf16 is
far too coarse. Aggregations (the big 4096-contraction GEMMs) use an
exact fp16 hi/lo split of s_in/s_out: A is 0/1 (exact in fp16), so
A@s = A@hi + (1/2048)*A@(2048*(s-hi)) recovers ~22 mantissa bits at
2 passes of full PE rate (2x faster than native fp32 matmul). The lo
part is pre-scaled by 2^11 to dodge fp16 subnormal flushing. All other
GEMMs run native fp32; sigmoid is computed as 0.5+0.5*tanh(x/2) (tanh
LUT is ~50x more accurate than the sigmoid LUT).
"""

import numpy as np

import concourse.bass as bass
import concourse.mybir as mybir
import concourse.tile as tile
from concourse import bacc
from concourse import bass_utils

N = 4096
D = 512
NCORES = 8
NS = N // NCORES          # 512 local nodes per core
NT = NS // 128            # 4 node tiles
DT = D // 128             # 4 feature tiles
CT = 3 * D // 128         # 12 concat-feature tiles
JT = N // 128             # 32 contraction tiles for aggregation
N_STEPS = 5
LO_SCALE = 2048.0         # 2^11: shift s-hi into fp16 normal range

F32 = mybir.dt.float32
F16 = mybir.dt.float16


def _build(n_steps=N_STEPS, use_collective=True):
    nc = bacc.Bacc("TRN2", target_bir_lowering=False, debug=False,
                   enable_asserts=True,
                   num_devices=NCORES if use_collective else 1)

    # ---- per-core external I/O ----
    state_t0 = nc.dram_tensor("state_t0", [D, NS], F32, kind="ExternalInput")
    a_in_rhs = nc.dram_tensor("a_in_rhs", [N, NS], F16, kind="ExternalInput")
    a_out_rhs = nc.dram_tensor("a_out_rhs", [N, NS], F16, kind="ExternalInput")
    in_wt = nc.dram_tensor("in_wt", [D, D], F32, kind="ExternalInput")
    out_wt = nc.dram_tensor("out_wt", [D, D], F32, kind="ExternalInput")
    # gate weights pre-chunked by output tile and hi/lo-split for the scaled
    # 3-pass f16 scheme: [mo, kc*128, 256] with cols 0:128 = f16 hi and
    # 128:256 = f16((w - hi) * LO_SCALE)
    r_wt = nc.dram_tensor("r_wt", [DT, 3 * D, 256], F16, kind="ExternalInput")
    z_wt = nc.dram_tensor("z_wt", [DT, 3 * D, 256], F16, kind="ExternalInput")
    h_wt = nc.dram_tensor("h_wt", [DT, 3 * D, 256], F16, kind="ExternalInput")
    o1_wt = nc.dram_tensor("o1_wt", [D, D], F32, kind="ExternalInput")
    o2_wt = nc.dram_tensor("o2_wt", [D, D], F32, kind="ExternalInput")
    in_b_row = nc.dram_tensor("in_b_row", [1, D], F32, kind="ExternalInput")
    out_b_row = nc.dram_tensor("out_b_row", [1, D], F32, kind="ExternalInput")
    o2_b_row = nc.dram_tensor("o2_b_row", [1, D], F32, kind="ExternalInput")
    rb_half = nc.dram_tensor("rb_half", [128, DT], F32, kind="ExternalInput")
    zb_half = nc.dram_tensor("zb_half", [128, DT], F32, kind="ExternalInput")
    hb_col = nc.dram_tensor("hb_col", [128, DT], F32, kind="ExternalInput")
    o1b_col = nc.dram_tensor("o1b_col", [128, DT], F32, kind="ExternalInput")
    out_shard = nc.dram_tensor("out_shard", [NS, D], F32, kind="ExternalOutput")

    with tile.TileContext(nc) as tc:
        with (
            tc.tile_pool(name="wpool", bufs=1) as wpool,
            tc.tile_pool(name="state", bufs=2) as spool,
            tc.tile_pool(name="gatew", bufs=3) as gwpool,
            tc.tile_pool(name="work", bufs=2) as work,
            tc.tile_pool(name="sjp", bufs=8) as sjp,
            tc.tile_pool(name="astp", bufs=6) as astp,
            tc.tile_pool(name="att", bufs=4) as att,
            tc.tile_pool(name="gout", bufs=1) as gout,
            tc.tile_pool(name="psum", bufs=8, space="PSUM") as psum,
            tc.tile_pool(name="dram", bufs=2, space="DRAM") as dram,
        ):
            # ---- resident weights / constants ----
            ain_sb = wpool.tile([128, JT, NS], F16, name="ain_sb")
            nc.sync.dma_start(ain_sb[:], a_in_rhs.ap().rearrange("(jt p) r -> p jt r", p=128))
            aout_dram = a_out_rhs.ap().rearrange("(jt p) r -> p jt r", p=128)
            inw_sb = wpool.tile([128, DT, D], F32, name="inw_sb")
            nc.sync.dma_start(inw_sb[:], in_wt.ap().rearrange("(ti p) o -> p ti o", p=128))
            outw_sb = wpool.tile([128, DT, D], F32, name="outw_sb")
            nc.sync.dma_start(outw_sb[:], out_wt.ap().rearrange("(ti p) o -> p ti o", p=128))
            inb_sb = wpool.tile([1, D], F32, name="inb_sb")
            nc.sync.dma_start(inb_sb[:], in_b_row.ap())
            outb_sb = wpool.tile([1, D], F32, name="outb_sb")
            nc.sync.dma_start(outb_sb[:], out_b_row.ap())
            o2b_sb = wpool.tile([1, D], F32, name="o2b_sb")
            nc.sync.dma_start(o2b_sb[:], o2_b_row.ap())
            rbh_sb = wpool.tile([128, DT], F32, name="rbh_sb")
            nc.sync.dma_start(rbh_sb[:], rb_half.ap())
            zbh_sb = wpool.tile([128, DT], F32, name="zbh_sb")
            nc.sync.dma_start(zbh_sb[:], zb_half.ap())
            hb_sb = wpool.tile([128, DT], F32, name="hb_sb")
            nc.sync.dma_start(hb_sb[:], hb_col.ap())
            o1b_sb = wpool.tile([128, DT], F32, name="o1b_sb")
            nc.sync.dma_start(o1b_sb[:], o1b_col.ap())
            ones_sb = wpool.tile([1, 128], F32, name="ones_sb")
            nc.vector.memset(ones_sb[:], 1.0)

            def split16(src, hi, lo):
                """hi = f16(src); lo = f16((src - hi) * LO_SCALE)."""
                nc.vector.tensor_copy(hi, src)
                dd = work.tile(list(src.shape), F32, name="dd", tag="dd")
                nc.vector.tensor_sub(dd[:], src, hi)
                nc.vector.tensor_scalar_mul(lo, dd[:], LO_SCALE)

            # ---- initial state (feature-major stateT [i, n]) ----
            st = spool.tile([128, DT, NS], F32, name="st")
            nc.sync.dma_start(st[:], state_t0.ap().rearrange("(ti p) n -> p ti n", p=128))
            st_hi = spool.tile([128, DT, NS], F16, name="st_hi", tag="sthi", bufs=1)
            st_lo = spool.tile([128, DT, NS], F16, name="st_lo", tag="stlo", bufs=1)
            for ti in range(DT):
                split16(st[:, ti, :], st_hi[:, ti, :], st_lo[:, ti, :])

            for step in range(n_steps):
                in_cc_a = dram.tile([NS, 2 * D], F16, name="in_cc_a", tag="icc")
                in_cc_b = dram.tile([NS, 2 * D], F16, name="in_cc_b", tag="icc")
                # s_in gathered per 128-row block (4 small AllGathers) so the
                # first aggregation matmuls can start while GEMM1 is still
                # running; s_out as one gather (it hides under a_in compute).
                out_ccs = [dram.tile([8 * 128, 2 * D], F16, name=f"occ_a{nt}",
                                     tag="occa", addr_space="Shared")
                           for nt in range(NT)]
                out_cc_b = dram.tile([N, 2 * D], F16, name="out_cc_b", tag="occ",
                                     addr_space="Shared")

                # ---- GEMM1: s_in / s_out (node-major [n, o]) + bias, f16 hi/lo
                for w_sb, b_sb, icc, blockwise in ((inw_sb, inb_sb, in_cc_a, True),
                                                   (outw_sb, outb_sb, in_cc_b, False)):
                    for nt in range(NT):
                        ps = psum.tile([128, D], F32, name="ps", tag="ps")
                        for ti in range(DT):
                            nc.tensor.matmul(ps[:], st[:, ti, nt * 128:(nt + 1) * 128],
                                             w_sb[:, ti, :], start=(ti == 0), stop=False)
                        nc.tensor.matmul(ps[:], ones_sb[:], b_sb[:], start=False, stop=True)
                        hi = work.tile([128, D], F16, name="hi", tag="hi")
                        nc.vector.tensor_copy(hi[:], ps[:])
                        dd = work.tile([128, D], F32, name="dd", tag="dd")
                        nc.vector.tensor_sub(dd[:], ps[:], hi[:])
                        lo = work.tile([128, D], F16, name="lo", tag="lo")
                        nc.vector.tensor_scalar_mul(lo[:], dd[:], LO_SCALE)
                        nc.sync.dma_start(icc[nt * 128:(nt + 1) * 128, 0:D], hi[:])
                        nc.sync.dma_start(icc[nt * 128:(nt + 1) * 128, D:2 * D], lo[:])
                        if blockwise and use_collective:
                            nc.gpsimd.collective_compute(
                                "AllGather",
                                mybir.AluOpType.bypass,
                                replica_groups=[list(range(NCORES))],
                                ins=[icc[nt * 128:(nt + 1) * 128, :].opt()],
                                outs=[out_ccs[nt].opt()],
                            )
                    if not blockwise:
                        if use_collective:
                            nc.gpsimd.collective_compute(
                                "AllGather",
                                mybir.AluOpType.bypass,
                                replica_groups=[list(range(NCORES))],
                                ins=[icc.opt()],
                                outs=[out_cc_b.opt()],
                            )
                        else:
                            nc.sync.dma_start(out_cc_b[0:NS, :], icc[:])
                    elif not use_collective:
                        for nt in range(NT):
                            nc.sync.dma_start(out_ccs[nt][0:128, :],
                                              in_cc_a[nt * 128:(nt + 1) * 128, :])

                occ_b = out_cc_b.rearrange("(jt p) c -> p jt c", p=128)

                # ---- aggregations: a_inT / a_outT (feature-major [f, r]),
                # evicted directly into f16 hi/lo pairs for the 3-pass gates
                ai_hi = att.tile([128, DT, NS], F16, name="ai_hi", tag="aT16")
                ai_lo = att.tile([128, DT, NS], F16, name="ai_lo", tag="aT16")
                ao_hi = att.tile([128, DT, NS], F16, name="ao_hi", tag="aT16")
                ao_lo = att.tile([128, DT, NS], F16, name="ao_lo", tag="aT16")

                # a_in: consume gather blocks in arrival order; block b core c
                # holds nodes j-tile c*NT + b
                ph = [psum.tile([128, NS], F32, name=f"ph{f}", tag="ps") for f in range(DT)]
                pl = [psum.tile([128, NS], F32, name=f"pl{f}", tag="ps") for f in range(DT)]
                for b in range(NT):
                    occ_bv = out_ccs[b].rearrange("(cc p) c -> p cc c", p=128)
                    for cc in range(NCORES):
                        jt = cc * NT + b
                        sj = sjp.tile([128, 2 * D], F16, name="sj", tag="sj")
                        nc.sync.dma_start(sj[:], occ_bv[:, cc, :])
                        first = (b == 0 and cc == 0)
                        last = (b == NT - 1 and cc == NCORES - 1)
                        for f in range(DT):
                            nc.tensor.matmul(ph[f][:], sj[:, f * 128:(f + 1) * 128],
                                             ain_sb[:, jt, :], start=first, stop=last)
                            nc.tensor.matmul(pl[f][:], sj[:, D + f * 128:D + (f + 1) * 128],
                                             ain_sb[:, jt, :], start=first, stop=last)
                for f in range(DT):
                    tmp = work.tile([128, NS], F32, name="tmph", tag="tmph")
                    nc.vector.tensor_copy(tmp[:], ph[f][:])
                    full = work.tile([128, NS], F32, name="aTf", tag="aTf")
                    nc.vector.scalar_tensor_tensor(
                        full[:], pl[f][:], 1.0 / LO_SCALE, tmp[:],
                        mybir.AluOpType.mult, mybir.AluOpType.add)
                    split16(full[:], ai_hi[:, f, :], ai_lo[:, f, :])

                # a_out
                ph = [psum.tile([128, NS], F32, name=f"qh{f}", tag="ps") for f in range(DT)]
                pl = [psum.tile([128, NS], F32, name=f"ql{f}", tag="ps") for f in range(DT)]
                for jt in range(JT):
                    sj = sjp.tile([128, 2 * D], F16, name="sj", tag="sj")
                    nc.sync.dma_start(sj[:], occ_b[:, jt, :])
                    ast = astp.tile([128, NS], F16, name="ast", tag="ast")
                    nc.sync.dma_start(ast[:], aout_dram[:, jt, :])
                    for f in range(DT):
                        nc.tensor.matmul(ph[f][:], sj[:, f * 128:(f + 1) * 128],
                                         ast[:], start=(jt == 0), stop=(jt == JT - 1))
                        nc.tensor.matmul(pl[f][:], sj[:, D + f * 128:D + (f + 1) * 128],
                                         ast[:], start=(jt == 0), stop=(jt == JT - 1))
                for f in range(DT):
                    tmp = work.tile([128, NS], F32, name="tmph", tag="tmph")
                    nc.vector.tensor_copy(tmp[:], ph[f][:])
                    full = work.tile([128, NS], F32, name="aTf", tag="aTf")
                    nc.vector.scalar_tensor_tensor(
                        full[:], pl[f][:], 1.0 / LO_SCALE, tmp[:],
                        mybir.AluOpType.mult, mybir.AluOpType.add)
                    split16(full[:], ao_hi[:, f, :], ao_lo[:, f, :])

                # ---- gates: scaled 3-pass f16 (hi@hi into ps_m; hi@lo + lo@hi
                # into ps_c, both carrying one factor of LO_SCALE) ----
                def a_rhs(c):
                    if c < DT:
                        return ai_hi[:, c, :], ai_lo[:, c, :]
                    if c < 2 * DT:
                        return ao_hi[:, c - DT, :], ao_lo[:, c - DT, :]
                    return st_hi[:, c - 2 * DT, :], st_lo[:, c - 2 * DT, :]

                rT = gout.tile([128, DT, NS], F32, name="rT", tag="rT")
                zT = gout.tile([128, DT, NS], F32, name="zT", tag="zT")
                rs_hi = gout.tile([128, DT, NS], F16, name="rs_hi", tag="rshi")
                rs_lo = gout.tile([128, DT, NS], F16, name="rs_lo", tag="rslo")

                def gate_pair(w_dram, rhs_fn, evict_fn, tilename):
                    for mo in range(DT):
                        gw = gwpool.tile([128, CT, 256], F16, name=tilename, tag="gw")
                        nc.sync.dma_start(
                            gw[:], w_dram.ap()[mo].rearrange("(kc p) m -> p kc m", p=128))
                        ps_m = psum.tile([128, NS], F32, name="psgm", tag="ps")
                        ps_c = psum.tile([128, NS], F32, name="psgc", tag="ps")
                        for c in range(CT):
                            hi, lo = rhs_fn(c)
                            nc.tensor.matmul(ps_m[:], gw[:, c, 0:128], hi,
                                             start=(c == 0), stop=(c == CT - 1))
                            nc.tensor.matmul(ps_c[:], gw[:, c, 128:256], hi,
                                             start=(c == 0), stop=False)
                            nc.tensor.matmul(ps_c[:], gw[:, c, 0:128], lo,
                                             start=False, stop=(c == CT - 1))
                        t1 = work.tile([128, NS], F32, name="tmph", tag="tmph")
                        nc.vector.tensor_copy(t1[:], ps_m[:])
                        pre = work.tile([128, NS], F32, name="aTf", tag="aTf")
                        nc.vector.scalar_tensor_tensor(
                            pre[:], ps_c[:], 1.0 / LO_SCALE, t1[:],
                            mybir.AluOpType.mult, mybir.AluOpType.add)
                        evict_fn(mo, pre)

                def evict_r(mo, pre):
                    # T_r = tanh(0.5*pre + 0.5*b); r = 0.5 + 0.5*T_r
                    nc.scalar.activation(rT[:, mo, :], pre[:], mybir.ActivationFunctionType.Tanh,
                                         bias=rbh_sb[:, mo:mo + 1], scale=0.5)
                    # rs = (0.5 + 0.5*T_r) * state, split for the h-gate
                    rfull = work.tile([128, NS], F32, name="rfull", tag="rfull")
                    nc.vector.tensor_scalar(rfull[:], rT[:, mo, :], 0.5, 0.5,
                                            mybir.AluOpType.mult, mybir.AluOpType.add)
                    rsf = work.tile([128, NS], F32, name="rsf", tag="rsf")
                    nc.vector.tensor_mul(rsf[:], rfull[:], st[:, mo, :])
                    split16(rsf[:], rs_hi[:, mo, :], rs_lo[:, mo, :])

                def evict_z(mo, pre):
                    nc.scalar.activation(zT[:, mo, :], pre[:], mybir.ActivationFunctionType.Tanh,
                                         bias=zbh_sb[:, mo:mo + 1], scale=0.5)

                gate_pair(r_wt, a_rhs, evict_r, "gw_r")
                gate_pair(z_wt, a_rhs, evict_z, "gw_z")

                def j_rhs(c):
                    if c < 2 * DT:
                        return a_rhs(c)
                    return rs_hi[:, c - 2 * DT, :], rs_lo[:, c - 2 * DT, :]

                st_new = spool.tile([128, DT, NS], F32, name="st")

                def evict_h(mo, pre):
                    hh = work.tile([128, NS], F32, name="hh", tag="hh")
                    nc.scalar.activation(hh[:], pre[:], mybir.ActivationFunctionType.Tanh,
                                         bias=hb_sb[:, mo:mo + 1], scale=1.0)
                    # state' = state + (0.5 + 0.5*T_z)*(h - state)
                    d2 = work.tile([128, NS], F32, name="d2", tag="d2")
                    nc.vector.tensor_sub(d2[:], hh[:], st[:, mo, :])
                    e2 = work.tile([128, NS], F32, name="e2", tag="e2")
                    nc.vector.tensor_mul(e2[:], zT[:, mo, :], d2[:])
                    g2 = work.tile([128, NS], F32, name="g2", tag="g2")
                    nc.vector.tensor_add(g2[:], d2[:], e2[:])
                    nc.vector.scalar_tensor_tensor(
                        st_new[:, mo, :], g2[:], 0.5, st[:, mo, :],
                        mybir.AluOpType.mult, mybir.AluOpType.add)
                    split16(st_new[:, mo, :], st_hi[:, mo, :], st_lo[:, mo, :])

                gate_pair(h_wt, j_rhs, evict_h, "gw_h")
                st = st_new

            # ---- output MLP ----
            o1w_sb = gwpool.tile([128, DT, D], F32, name="o1w_sb", tag="gw")
            nc.sync.dma_start(o1w_sb[:], o1_wt.ap().rearrange("(ti p) o -> p ti o", p=128))
            o2w_sb = gwpool.tile([128, DT, D], F32, name="o2w_sb", tag="gw")
            nc.sync.dma_start(o2w_sb[:], o2_wt.ap().rearrange("(ti p) o -> p ti o", p=128))

            tT = gout.tile([128, DT, NS], F32, name="tT", tag="rT")
            pst = [psum.tile([128, NS], F32, name=f"ps_t{mo}", tag="ps") for mo in range(DT)]
            for ti in range(DT):
                for mo in range(DT):
                    nc.tensor.matmul(pst[mo][:], o1w_sb[:, ti, mo * 128:(mo + 1) * 128],
                                     st[:, ti, :], start=(ti == 0), stop=(ti == DT - 1))
            for mo in range(DT):
                nc.scalar.activation(tT[:, mo, :], pst[mo][:], mybir.ActivationFunctionType.Tanh,
                                     bias=o1b_sb[:, mo:mo + 1], scale=1.0)

            out_sb = gout.tile([128, NT, D], F32, name="out_sb", tag="zT")
            pso = [psum.tile([128, D], F32, name=f"ps_o{nt}", tag="ps") for nt in range(NT)]
            for c in range(DT):
                for nt in range(NT):
                    nc.tensor.matmul(pso[nt][:], tT[:, c, nt * 128:(nt + 1) * 128],
                                     o2w_sb[:, c, :], start=(c == 0), stop=False)
            for nt in range(NT):
                nc.tensor.matmul(pso[nt][:], ones_sb[:], o2b_sb[:], start=False, stop=True)
                nc.vector.tensor_copy(out_sb[:, nt, :], pso[nt][:])
            nc.sync.dma_start(out_shard.ap().rearrange("(nt p) o -> p nt o", p=128), out_sb[:])

    nc.compile()
    return nc


_NC_CACHE = {}


def _get_nc(n_steps=N_STEPS):
    if n_steps not in _NC_CACHE:
        _NC_CACHE[n_steps] = _build(n_steps)
    return _NC_CACHE[n_steps]


def _prep_in_maps(prop_state, A, in_W, in_b, out_W, out_b, r_W, r_b,
                  z_W, z_b, h_W, h_b, o1_W, o1_b, o2_W, o2_b):
    Af = np.ascontiguousarray(A).astype(np.float32)
    f32 = np.float32

    def rep(x):
        return np.ascontiguousarray(x, dtype=f32)

    def gate_w(W):
        # W.T is [3D, D]; chunk into [mo, 3D, 128], then hi/lo split into
        # [mo, 3D, 256] f16 (cols 0:128 hi, 128:256 (w-hi)*LO_SCALE)
        wt = W.T.astype(f32).reshape(3 * D, DT, 128).transpose(1, 0, 2)
        hi = wt.astype(np.float16)
        lo = ((wt - hi.astype(f32)) * LO_SCALE).astype(np.float16)
        return np.ascontiguousarray(np.concatenate([hi, lo], axis=-1))

    shared = {
        "in_wt": rep(in_W.T), "out_wt": rep(out_W.T),
        "r_wt": gate_w(r_W), "z_wt": gate_w(z_W), "h_wt": gate_w(h_W),
        "o1_wt": rep(o1_W.T), "o2_wt": rep(o2_W.T),
        "in_b_row": rep(in_b).reshape(1, D), "out_b_row": rep(out_b).reshape(1, D),
        "o2_b_row": rep(o2_b).reshape(1, D),
        "rb_half": rep(r_b / 2).reshape(DT, 128).T.copy(),
        "zb_half": rep(z_b / 2).reshape(DT, 128).T.copy(),
        "hb_col": rep(h_b).reshape(DT, 128).T.copy(),
        "o1b_col": rep(o1_b).reshape(DT, 128).T.copy(),
    }
    in_maps = []
    for k in range(NCORES):
        rows = slice(k * NS, (k + 1) * NS)
        m = dict(shared)
        m["state_t0"] = np.ascontiguousarray(prop_state[rows].astype(f32).T)
        m["a_in_rhs"] = np.ascontiguousarray(Af[rows, :].T.astype(np.float16))
        m["a_out_rhs"] = np.ascontiguousarray(Af[:, rows].astype(np.float16))
        in_maps.append(m)
    return in_maps


def run(trace=False, **inputs):
    nc = _get_nc()
    in_maps = _prep_in_maps(**inputs)
    res = bass_utils.run_bass_kernel_spmd(
        nc, in_maps, core_ids=list(range(NCORES)), trace=trace)
    out = np.concatenate([res.results[k]["out_shard"] for k in range(NCORES)], axis=0)
    return out, res


def kernel(**inputs) -> np.ndarray:
    out, _ = run(trace=False, **inputs)
    return out


# revision 34
# speedup vs baseline: 1.2238x; 1.2238x over previous
"""GGNN (gated graph NN) message-passing kernel for Trainium2, 8 NeuronCores.

Model (per reference):
  5 steps of: s_in = state @ in_W.T + in_b ; s_out = state @ out_W.T + out_b
              a_in = A @ s_in ; a_out = A.T @ s_out
              r = sigmoid([a_in, a_out, state] @ r_W.T + r_b)
              z = sigmoid([a_in, a_out, state] @ z_W.T + z_b)
              h = tanh([a_in, a_out, r*state] @ h_W.T + h_b)
              state = (1-z)*state + z*h
  out = tanh(state @ o1_W.T + o1_b) @ o2_W.T + o2_b

Sharding: 1D node parallelism over 8 cores (512 nodes each). Weights
replicated. Per step, each core computes its s_in/s_out shard, all-gathers
them, then computes its row-shard of the aggregations and gates locally.

Precision: the recurrence amplifies operand rounding ~2500x, so bf16 is
far too coarse. Aggregations (the big 4096-contraction GEMMs) use an
exact fp16 hi/lo split of s_in/s_out: A is 0/1 (exact in fp16), so
A@s = A@hi + (1/2048)*A@(2048*(s-hi)) recovers ~22 mantissa bits at
2 passes of full PE rate (2x faster than native fp32 matmul). The lo
part is pre-scaled by 2^11 to dodge fp16 subnormal flushing. All other
GEMMs run native fp32; sigmoid is computed as 0.5+0.5*tanh(x/2) (tanh
LUT is ~50x more accurate than the sigmoid LUT).
"""

import numpy as np

import concourse.bass as bass
import concourse.mybir as mybir
import concourse.tile as tile
from concourse import bacc
from concourse import bass_utils

N = 4096
D = 512
NCORES = 8
NS = N // NCORES          # 512 local nodes per core
NT = NS // 128            # 4 node tiles
DT = D // 128             # 4 feature tiles
CT = 3 * D // 128         # 12 concat-feature tiles
JT = N // 128             # 32 contraction tiles for aggregation
N_STEPS = 5
LO_SCALE = 2048.0         # 2^11: shift s-hi into fp16 normal range

F32 = mybir.dt.float32
F16 = mybir.dt.float16


def _build(n_steps=N_STEPS, use_collective=True):
    nc = bacc.Bacc("TRN2", target_bir_lowering=False, debug=False,
                   enable_asserts=True,
                   num_devices=NCORES if use_collective else 1)

    # ---- per-core external I/O ----
    state_t0 = nc.dram_tensor("state_t0", [D, NS], F32, kind="ExternalInput")
    a_in_rhs = nc.dram_tensor("a_in_rhs", [N, NS], F16, kind="ExternalInput")
    a_out_rhs = nc.dram_tensor("a_out_rhs", [N, NS], F16, kind="ExternalInput")
    in_wt = nc.dram_tensor("in_wt", [D, D], F32, kind="ExternalInput")
    out_wt = nc.dram_tensor("out_wt", [D, D], F32, kind="ExternalInput")
    # gate weights pre-chunked by output tile and hi/lo-split for the scaled
    # 3-pass f16 scheme: [mo, kc*128, 256] with cols 0:128 = f16 hi and
    # 128:256 = f16((w - hi) * LO_SCALE)
    r_wt = nc.dram_tensor("r_wt", [DT, 3 * D, 256], F16, kind="ExternalInput")
    z_wt = nc.dram_tensor("z_wt", [DT, 3 * D, 256], F16, kind="ExternalInput")
    h_wt = nc.dram_tensor("h_wt", [DT, 3 * D, 256], F16, kind="ExternalInput")
    o1_wt = nc.dram_tensor("o1_wt", [D, D], F32, kind="ExternalInput")
    o2_wt = nc.dram_tensor("o2_wt", [D, D], F32, kind="ExternalInput")
    in_b_row = nc.dram_tensor("in_b_row", [1, D], F32, kind="ExternalInput")
    out_b_row = nc.dram_tensor("out_b_row", [1, D], F32, kind="ExternalInput")
    o2_b_row = nc.dram_tensor("o2_b_row", [1, D], F32, kind="ExternalInput")
    rb_half = nc.dram_tensor("rb_half", [128, DT], F32, kind="ExternalInput")
    zb_half = nc.dram_tensor("zb_half", [128, DT], F32, kind="ExternalInput")
    hb_col = nc.dram_tensor("hb_col", [128, DT], F32, kind="ExternalInput")
    o1b_col = nc.dram_tensor("o1b_col", [128, DT], F32, kind="ExternalInput")
    out_shard = nc.dram_tensor("out_shard", [NS, D], F32, kind="ExternalOutput")

    with tile.TileContext(nc) as tc:
        with (
            tc.tile_pool(name="wpool", bufs=1) as wpool,
            tc.tile_pool(name="state", bufs=2) as spool,
            tc.tile_pool(name="gatew", bufs=3) as gwpool,
            tc.tile_pool(name="work", bufs=2) as work,
            tc.tile_pool(name="sjp", bufs=8) as sjp,
            tc.tile_pool(name="astp", bufs=6) as astp,
            tc.tile_pool(name="att", bufs=4) as att,
            tc.tile_pool(name="gout", bufs=1) as gout,
            tc.tile_pool(name="psum", bufs=8, space="PSUM") as psum,
            tc.tile_pool(name="dram", bufs=2, space="DRAM") as dram,
        ):
            # ---- resident weights / constants ----
            ain_sb = wpool.tile([128, JT, NS], F16, name="ain_sb")
            nc.sync.dma_start(ain_sb[:], a_in_rhs.ap().rearrange("(jt p) r -> p jt r", p=128))
            aout_dram = a_out_rhs.ap().rearrange("(jt p) r -> p jt r", p=128)
            inw_sb = wpool.tile([128, DT, D], F32, name="inw_sb")
            nc.sync.dma_start(inw_sb[:], in_wt.ap().rearrange("(ti p) o -> p ti o", p=128))
            outw_sb = wpool.tile([128, DT, D], F32, name="outw_sb")
            nc.sync.dma_start(outw_sb[:], out_wt.ap().rearrange("(ti p) o -> p ti o", p=128))
            inb_sb = wpool.tile([1, D], F32, name="inb_sb")
            nc.sync.dma_start(inb_sb[:], in_b_row.ap())
            outb_sb = wpool.tile([1, D], F32, name="outb_sb")
            nc.sync.dma_start(outb_sb[:], out_b_row.ap())
            o2b_sb = wpool.tile([1, D], F32, name="o2b_sb")
            nc.sync.dma_start(o2b_sb[:], o2_b_row.ap())
            rbh_sb = wpool.tile([128, DT], F32, name="rbh_sb")
            nc.sync.dma_start(rbh_sb[:], rb_half.ap())
            zbh_sb = wpool.tile([128, DT], F32, name="zbh_sb")
            nc.sync.dma_start(zbh_sb[:], zb_half.ap())
            hb_sb = wpool.tile([128, DT], F32, name="hb_sb")
            nc.sync.dma_start(hb_sb[:], hb_col.ap())
            o1b_sb = wpool.tile([128, DT], F32, name="o1b_sb")
            nc.sync.dma_start(o1b_sb[:], o1b_col.ap())
            ones_sb = wpool.tile([1, 128], F32, name="ones_sb")
            nc.vector.memset(ones_sb[:], 1.0)

            def split16(src, hi, lo):
                """hi = f16(src); lo = f16((src - hi) * LO_SCALE)."""
                nc.vector.tensor_copy(hi, src)
                dd = work.tile(list(src.shape), F32, name="dd", tag="dd")
                nc.vector.tensor_sub(dd[:], src, hi)
                nc.vector.tensor_scalar_mul(lo, dd[:], LO_SCALE)

            # ---- initial state (feature-major stateT [i, n]) ----
            st = spool.tile([128, DT, NS], F32, name="st")
            nc.sync.dma_start(st[:], state_t0.ap().rearrange("(ti p) n -> p ti n", p=128))
            st_hi = spool.tile([128, DT, NS], F16, name="st_hi", tag="sthi", bufs=1)
            st_lo = spool.tile([128, DT, NS], F16, name="st_lo", tag="stlo", bufs=1)
            for ti in range(DT):
                split16(st[:, ti, :], st_hi[:, ti, :], st_lo[:, ti, :])

            for step in range(n_steps):
                in_cc_a = dram.tile([NS, 2 * D], F16, name="in_cc_a", tag="icc")
                in_cc_b = dram.tile([NS, 2 * D], F16, name="in_cc_b", tag="icc")
                # s_in and s_out gathered per 128-row block (8 small
                # AllGathers): the first aggregation matmuls start while GEMM1
                # is still running, and the gather's SDMA traffic is spread
                # into the DMA-light GEMM1 window
                out_ccs_a = [dram.tile([8 * 128, 2 * D], F16, name=f"occ_a{nt}",
                                       tag="occa", addr_space="Shared")
                             for nt in range(NT)]
                out_ccs_b = [dram.tile([8 * 128, 2 * D], F16, name=f"occ_b{nt}",
                                       tag="occb", addr_space="Shared")
                             for nt in range(NT)]

                # ---- GEMM1: s_in / s_out (node-major [n, o]) + bias, f16 hi/lo
                for w_sb, b_sb, icc, occ_ts in ((inw_sb, inb_sb, in_cc_a, out_ccs_a),
                                                (outw_sb, outb_sb, in_cc_b, out_ccs_b)):
                    for nt in range(NT):
                        ps = psum.tile([128, D], F32, name="ps", tag="ps")
                        for ti in range(DT):
                            nc.tensor.matmul(ps[:], st[:, ti, nt * 128:(nt + 1) * 128],
                                             w_sb[:, ti, :], start=(ti == 0), stop=False)
                        nc.tensor.matmul(ps[:], ones_sb[:], b_sb[:], start=False, stop=True)
                        hi = work.tile([128, D], F16, name="hi", tag="hi")
                        nc.vector.tensor_copy(hi[:], ps[:])
                        dd = work.tile([128, D], F32, name="dd", tag="dd")
                        nc.vector.tensor_sub(dd[:], ps[:], hi[:])
                        lo = work.tile([128, D], F16, name="lo", tag="lo")
                        nc.vector.tensor_scalar_mul(lo[:], dd[:], LO_SCALE)
                        nc.sync.dma_start(icc[nt * 128:(nt + 1) * 128, 0:D], hi[:])
                        nc.sync.dma_start(icc[nt * 128:(nt + 1) * 128, D:2 * D], lo[:])
                        if use_collective:
                            nc.gpsimd.collective_compute(
                                "AllGather",
                                mybir.AluOpType.bypass,
                                replica_groups=[list(range(NCORES))],
                                ins=[icc[nt * 128:(nt + 1) * 128, :].opt()],
                                outs=[occ_ts[nt].opt()],
                            )
                        else:
                            nc.sync.dma_start(occ_ts[nt][0:128, :],
                                              icc[nt * 128:(nt + 1) * 128, :])

                # ---- aggregations: a_inT / a_outT (feature-major [f, r]),
                # evicted directly into f16 hi/lo pairs for the 3-pass gates
                ai_hi = att.tile([128, DT, NS], F16, name="ai_hi", tag="aT16")
                ai_lo = att.tile([128, DT, NS], F16, name="ai_lo", tag="aT16")
                ao_hi = att.tile([128, DT, NS], F16, name="ao_hi", tag="aT16")
                ao_lo = att.tile([128, DT, NS], F16, name="ao_lo", tag="aT16")

                # consume gather blocks in arrival order; block b of core c
                # holds nodes j-tile c*NT + b
                for occ_ts, a_src, hi_t, lo_t in ((out_ccs_a, "ain", ai_hi, ai_lo),
                                                  (out_ccs_b, "aout", ao_hi, ao_lo)):
                    ph = [psum.tile([128, NS], F32, name=f"ph{f}", tag="ps")
                          for f in range(DT)]
                    pl = [psum.tile([128, NS], F32, name=f"pl{f}", tag="ps")
                          for f in range(DT)]
                    for b in range(NT):
                        occ_bv = occ_ts[b].rearrange("(cc p) c -> p cc c", p=128)
                        for cc in range(NCORES):
                            jt = cc * NT + b
                            sj = sjp.tile([128, 2 * D], F16, name="sj", tag="sj")
                            nc.sync.dma_start(sj[:], occ_bv[:, cc, :])
                            if a_src == "ain":
                                a_rhs_tile = ain_sb[:, jt, :]
                            else:
                                ast = astp.tile([128, NS], F16, name="ast", tag="ast")
                                nc.sync.dma_start(ast[:], aout_dram[:, jt, :])
                                a_rhs_tile = ast[:]
                            first = (b == 0 and cc == 0)
                            last = (b == NT - 1 and cc == NCORES - 1)
                            for f in range(DT):
                                nc.tensor.matmul(ph[f][:], sj[:, f * 128:(f + 1) * 128],
                                                 a_rhs_tile, start=first, stop=last)
                                nc.tensor.matmul(pl[f][:], sj[:, D + f * 128:D + (f + 1) * 128],
                                                 a_rhs_tile, start=first, stop=last)
                    for f in range(DT):
                        tmp = work.tile([128, NS], F32, name="tmph", tag="tmph")
                        nc.vector.tensor_copy(tmp[:], ph[f][:])
                        full = work.tile([128, NS], F32, name="aTf", tag="aTf")
                        nc.vector.scalar_tensor_tensor(
                            full[:], pl[f][:], 1.0 / LO_SCALE, tmp[:],
                            mybir.AluOpType.mult, mybir.AluOpType.add)
                        split16(full[:], hi_t[:, f, :], lo_t[:, f, :])

                # ---- gates: scaled 3-pass f16 (hi@hi into ps_m; hi@lo + lo@hi
                # into ps_c, both carrying one factor of LO_SCALE) ----
                def a_rhs(c):
                    if c < DT:
                        return ai_hi[:, c, :], ai_lo[:, c, :]
                    if c < 2 * DT:
                        return ao_hi[:, c - DT, :], ao_lo[:, c - DT, :]
                    return st_hi[:, c - 2 * DT, :], st_lo[:, c - 2 * DT, :]

                rT = gout.tile([128, DT, NS], F32, name="rT", tag="rT")
                zT = gout.tile([128, DT, NS], F32, name="zT", tag="zT")
                rs_hi = gout.tile([128, DT, NS], F16, name="rs_hi", tag="rshi")
                rs_lo = gout.tile([128, DT, NS], F16, name="rs_lo", tag="rslo")

                def gate_pair(w_dram, rhs_fn, evict_fn, tilename):
                    for mo in range(DT):
                        gw = gwpool.tile([128, CT, 256], F16, name=tilename, tag="gw")
                        nc.sync.dma_start(
                            gw[:], w_dram.ap()[mo].rearrange("(kc p) m -> p kc m", p=128))
                        ps_m = psum.tile([128, NS], F32, name="psgm", tag="ps")
                        ps_c = psum.tile([128, NS], F32, name="psgc", tag="ps")
                        for c in range(CT):
                            hi, lo = rhs_fn(c)
                            nc.tensor.matmul(ps_m[:], gw[:, c, 0:128], hi,
                                             start=(c == 0), stop=(c == CT - 1))
                            nc.tensor.matmul(ps_c[:], gw[:, c, 128:256], hi,
                                             start=(c == 0), stop=False)
                            nc.tensor.matmul(ps_c[:], gw[:, c, 0:128], lo,
                                             start=False, stop=(c == CT - 1))
                        t1 = work.tile([128, NS], F32, name="tmph", tag="tmph")
                        nc.vector.tensor_copy(t1[:], ps_m[:])
                        pre = work.tile([128, NS], F32, name="aTf", tag="aTf")
                        nc.vector.scalar_tensor_tensor(
                            pre[:], ps_c[:], 1.0 / LO_SCALE, t1[:],
                            mybir.AluOpType.mult, mybir.AluOpType.add)
                        evict_fn(mo, pre)

                def evict_r(mo, pre):
                    # T_r = tanh(0.5*pre + 0.5*b); r = 0.5 + 0.5*T_r
                    nc.scalar.activation(rT[:, mo, :], pre[:], mybir.ActivationFunctionType.Tanh,
                                         bias=rbh_sb[:, mo:mo + 1], scale=0.5)
                    # rs = (0.5 + 0.5*T_r) * state, split for the h-gate
                    rfull = work.tile([128, NS], F32, name="rfull", tag="rfull")
                    nc.vector.tensor_scalar(rfull[:], rT[:, mo, :], 0.5, 0.5,
                                            mybir.AluOpType.mult, mybir.AluOpType.add)
                    rsf = work.tile([128, NS], F32, name="rsf", tag="rsf")
                    nc.vector.tensor_mul(rsf[:], rfull[:], st[:, mo, :])
                    split16(rsf[:], rs_hi[:, mo, :], rs_lo[:, mo, :])

                def evict_z(mo, pre):
                    nc.scalar.activation(zT[:, mo, :], pre[:], mybir.ActivationFunctionType.Tanh,
                                         bias=zbh_sb[:, mo:mo + 1], scale=0.5)

                gate_pair(r_wt, a_rhs, evict_r, "gw_r")
                gate_pair(z_wt, a_rhs, evict_z, "gw_z")

                def j_rhs(c):
                    if c < 2 * DT:
                        return a_rhs(c)
                    return rs_hi[:, c - 2 * DT, :], rs_lo[:, c - 2 * DT, :]

                st_new = spool.tile([128, DT, NS], F32, name="st")

                def evict_h(mo, pre):
                    hh = work.tile([128, NS], F32, name="hh", tag="hh")
                    nc.scalar.activation(hh[:], pre[:], mybir.ActivationFunctionType.Tanh,
                                         bias=hb_sb[:, mo:mo + 1], scale=1.0)
                    # state' = state + (0.5 + 0.5*T_z)*(h - state)
                    d2 = work.tile([128, NS], F32, name="d2", tag="d2")
                    nc.vector.tensor_sub(d2[:], hh[:], st[:, mo, :])
                    e2 = work.tile([128, NS], F32, name="e2", tag="e2")
                    nc.vector.tensor_mul(e2[:], zT[:, mo, :], d2[:])
                    g2 = work.tile([128, NS], F32, name="g2", tag="g2")
                    nc.vector.tensor_add(g2[:], d2[:], e2[:])
                    nc.vector.scalar_tensor_tensor(
                        st_new[:, mo, :], g2[:], 0.5, st[:, mo, :],
                        mybir.AluOpType.mult, mybir.AluOpType.add)
                    split16(st_new[:, mo, :], st_hi[:, mo, :], st_lo[:, mo, :])

                gate_pair(h_wt, j_rhs, evict_h, "gw_h")
                st = st_new

            # ---- output MLP ----
            o1w_sb = gwpool.tile([128, DT, D], F32, name="o1w_sb", tag="gw")
            nc.sync.dma_start(o1w_sb[:], o1_wt.ap().rearrange("(ti p) o -> p ti o", p=128))
            o2w_sb = gwpool.tile([128, DT, D], F32, name="o2w_sb", tag="gw")
            nc.sync.dma_start(o2w_sb[:], o2_wt.ap().rearrange("(ti p) o -> p ti o", p=128))

            tT = gout.tile([128, DT, NS], F32, name="tT", tag="rT")
            pst = [psum.tile([128, NS], F32, name=f"ps_t{mo}", tag="ps") for mo in range(DT)]
            for ti in range(DT):
                for mo in range(DT):
                    nc.tensor.matmul(pst[mo][:], o1w_sb[:, ti, mo * 128:(mo + 1) * 128],
                                     st[:, ti, :], start=(ti == 0), stop=(ti == DT - 1))
            for mo in range(DT):
                nc.scalar.activation(tT[:, mo, :], pst[mo][:], mybir.ActivationFunctionType.Tanh,
                                     bias=o1b_sb[:, mo:mo + 1], scale=1.0)

            out_sb = gout.tile([128, NT, D], F32, name="out_sb", tag="zT")
            pso = [psum.tile([128, D], F32, name=f"ps_o{nt}", tag="ps") for nt in range(NT)]
            for c in range(DT):
                for nt in range(NT):
                    nc.tensor.matmul(pso[nt][:], tT[:, c, nt * 128:(nt + 1) * 128],
                                     o2w_sb[:, c, :], start=(c == 0), stop=False)
            for nt in range(NT):
                nc.tensor.matmul(pso[nt][:], ones_sb[:], o2b_sb[:], start=False, stop=True)
                nc.vector.tensor_copy(out_sb[:, nt, :], pso[nt][:])
            nc.sync.dma_start(out_shard.ap().rearrange("(nt p) o -> p nt o", p=128), out_sb[:])

    nc.compile()
    return nc


_NC_CACHE = {}


def _get_nc(n_steps=N_STEPS):
    if n_steps not in _NC_CACHE:
        _NC_CACHE[n_steps] = _build(n_steps)
    return _NC_CACHE[n_steps]


def _prep_in_maps(prop_state, A, in_W, in_b, out_W, out_b, r_W, r_b,
                  z_W, z_b, h_W, h_b, o1_W, o1_b, o2_W, o2_b):
    Af = np.ascontiguousarray(A).astype(np.float32)
    f32 = np.float32

    def rep(x):
        return np.ascontiguousarray(x, dtype=f32)

    def gate_w(W):
        # W.T is [3D, D]; chunk into [mo, 3D, 128], then hi/lo split into
        # [mo, 3D, 256] f16 (cols 0:128 hi, 128:256 (w-hi)*LO_SCALE)
        wt = W.T.astype(f32).reshape(3 * D, DT, 128).transpose(1, 0, 2)
        hi = wt.astype(np.float16)
        lo = ((wt - hi.astype(f32)) * LO_SCALE).astype(np.float16)
        return np.ascontiguousarray(np.concatenate([hi, lo], axis=-1))

    shared = {
        "in_wt": rep(in_W.T), "out_wt": rep(out_W.T),
        "r_wt": gate_w(r_W), "z_wt": gate_w(z_W), "h_wt": gate_w(h_W),
        "o1_wt": rep(o1_W.T), "o2_wt": rep(o2_W.T),
        "in_b_row": rep(in_b).reshape(1, D), "out_b_row": rep(out_b).reshape(1, D),
        "o2_b_row": rep(o2_b).reshape(1, D),
        "rb_half": rep(r_b / 2).reshape(DT, 128).T.copy(),
        "zb_half": rep(z_b / 2).reshape(DT, 128).T.copy(),
        "hb_col": rep(h_b).reshape(DT, 128).T.copy(),
        "o1b_col": rep(o1_b).reshape(DT, 128).T.copy(),
    }
    in_maps = []
    for k in range(NCORES):
        rows = slice(k * NS, (k + 1) * NS)
        m = dict(shared)
        m["state_t0"] = np.ascontiguousarray(prop_state[rows].astype(f32).T)
        m["a_in_rhs"] = np.ascontiguousarray(Af[rows, :].T.astype(np.float16))
        m["a_out_rhs"] = np.ascontiguousarray(Af[:, rows].astype(np.float16))
        in_maps.append(m)
    return in_maps


def run(trace=False, **inputs):
    nc = _get_nc()
    in_maps = _prep_in_maps(**inputs)
    res = bass_utils.run_bass_kernel_spmd(
        nc, in_maps, core_ids=list(range(NCORES)), trace=trace)
    out = np.concatenate([res.results[k]["out_shard"] for k in range(NCORES)], axis=0)
    return out, res


def kernel(**inputs) -> np.ndarray:
    out, _ = run(trace=False, **inputs)
    return out
